# revision 5
# baseline (speedup 1.0000x reference)
"""Trainium2 Bass kernel for nn_CriticNetwork (gnn_message_passing).

Math: the reference GNN does mean-aggregation over a complete graph with
self-loops, so every node of an env sees the identical per-env mean.  The
whole network collapses to per-env scalars:

  m_b  = mean over the 16 nodes of obs[b]                      [128]
  p_b  = relu(m_b @ W1 + b1) @ W2 + b2                         [64]
  a_b  = p_b . (Wfc @ (Wattn[:64] + Wattn[64:]))               scalar
  w_b  = sigmoid(leaky_relu(a_b, 0.01))                        scalar
  c_b  = p_b . Wv[:64] + bv                                    scalar
  Q_bj = (act[b,j]-pi[b,j]) . Wvy ;  (Wvy = Wv[64:72])
  PS_b = sum_j pi[b,j].Wvy ;  QS_b = sum_j Q_bj
  xv[b,j] = c_b + (PS_b + w_b*QS_b)/16 - (w_b/16)*Q_bj
  out x[b*16+d, j] = xv[b,j]   (independent of d)
  out w[b*16+d, j] = w_b

Sharding: data-parallel over envs, 512 envs per core x 8 cores.

Per-core layout: local env e = 128*g + p (g = group, p = partition), so a
group's per-env scalars live one-per-partition.

Engine plan (v2):
  - inputs split across BOTH HWDGE rings (sync + scalar), ~2.5 MB each:
    each ring sustains only ~180 GB/s (SDMA ports are 2:1 muxed with the
    sibling NeuronCore), so balancing the rings doubles stream rate.
  - per-group node-sum: ONE strided DVE tensor_reduce (f outer, node
    inner) instead of a 4-op pairwise tree.
  - pol/act dot-product block (Q64/QS4/PS4) runs on the otherwise-idle
    GpSimd engine during the obs stream.
  - combine + output-broadcast + output DMA per group, so early groups'
    outputs stream while later obs groups still load.
"""

import numpy as np
from contextlib import ExitStack

import concourse.bass as bass
import concourse.bacc as bacc
import concourse.tile as tile
from concourse import mybir
from concourse.bass_utils import run_bass_kernel_spmd

B, N, A = 4096, 16, 8
D_IN, H1, DP, DZ = 128, 64, 64, 64
NCORES = 8
BC = B // NCORES          # 512 envs per core
RC = BC * N               # 8192 obs rows per core
G = 4                     # env groups per core
GE = BC // G              # 128 envs per group
CW = 272                  # const tile width

F32 = mybir.dt.float32
ALU = mybir.AluOpType
AFT = mybir.ActivationFunctionType


def _build():
    nc = bacc.Bacc("TRN2", target_bir_lowering=False, debug=False)

    obs = nc.dram_tensor("obs", [RC, D_IN], F32, kind="ExternalInput")
    pol = nc.dram_tensor("pol", [RC, A], F32, kind="ExternalInput")
    act = nc.dram_tensor("act", [RC, A], F32, kind="ExternalInput")
    cst = nc.dram_tensor("cst", [128, CW], F32, kind="ExternalInput")
    xo = nc.dram_tensor("xo", [RC, N], F32, kind="ExternalOutput")
    wo = nc.dram_tensor("wo", [RC, N], F32, kind="ExternalOutput")

    with ExitStack() as ctx:
        tc = ctx.enter_context(tile.TileContext(nc))
        consts = ctx.enter_context(tc.tile_pool(name="consts", bufs=1))
        obsp = ctx.enter_context(tc.tile_pool(name="obsp", bufs=4))
        pap = ctx.enter_context(tc.tile_pool(name="pap", bufs=1))
        gsb = ctx.enter_context(tc.tile_pool(name="gsb", bufs=1))
        sb = ctx.enter_context(tc.tile_pool(name="sb", bufs=2))
        outp = ctx.enter_context(tc.tile_pool(name="outp", bufs=2))
        pmtp = ctx.enter_context(tc.tile_pool(name="pmtp", bufs=2, space="PSUM"))
        php = ctx.enter_context(tc.tile_pool(name="php", bufs=2, space="PSUM"))
        pacp = ctx.enter_context(tc.tile_pool(name="pacp", bufs=2, space="PSUM"))
        pwtp = ctx.enter_context(tc.tile_pool(name="pwtp", bufs=2, space="PSUM"))

        # cst first on the scalar ring (small; needed by the first chain)
        cst_sb = consts.tile([128, CW], F32)
        nc.scalar.dma_start(out=cst_sb, in_=cst.ap())
        wvy8_sb = cst_sb[:, 0:8]            # Wvy on all partitions
        w1q_sb = cst_sb[:, 8:72]            # W1 / 16
        wq_sb = cst_sb[0:64, 72:74]         # W2 @ [wa | Wv[:64]]
        b1_sb = cst_sb[0:64, 138:139]
        biasq_sb = cst_sb[0:2, 140:141]     # [b2.wa, b2.Wv64 + bv]
        id2_sb = cst_sb[0:2, 142:144]       # eye(2)
        id128_sb = cst_sb[:, 144:272]       # eye(128)

        # pol/act with interleaved env layout: partition p, block g = env 128g+p
        pa_view = lambda t: t.ap().rearrange("(g p n) a -> p g (n a)", p=128, n=16)
        pol_sb = pap.tile([128, G, N * A], F32)
        nc.sync.dma_start(out=pol_sb, in_=pa_view(pol))          # ring A
        act_sb = pap.tile([128, G, N * A], F32)
        nc.scalar.dma_start(out=act_sb, in_=pa_view(act))        # ring B

        # preload the sigmoid ACT table while DMAs stream
        warm = consts.tile([1, 1], F32)
        nc.scalar.activation(out=warm, in_=cst_sb[0:1, 0:1], func=AFT.Sigmoid)

        # obs rows for env e=128g+p: 16e..16e+15 -> group g tile [128, 2048]
        # groups alternate rings: g0,g2 on sync (A), g1,g3 on scalar (B)
        obs_v = obs.ap().rearrange("(g p nf) f -> g p (nf f)", p=128, nf=16)
        obs_tiles = []
        for g in range(G):
            obs_t = obsp.tile([128, 16 * 128], F32, name="obs_t")
            eng = nc.sync if g % 2 == 0 else nc.scalar
            eng.dma_start(out=obs_t, in_=obs_v[g])
            obs_tiles.append(obs_t)

        # ---- GpSimd: pol/act dot block (runs during the obs stream) ----
        pol4 = pol_sb.rearrange("p g (r a) -> p g r a", a=8)
        act4 = act_sb.rearrange("p g (r a) -> p g r a", a=8)
        wvyb4 = wvy8_sb.unsqueeze(1).unsqueeze(1).broadcast_to([128, G, N, A])
        wvyb3 = wvy8_sb.unsqueeze(1).broadcast_to([128, G, A])

        # big elementwise on GpSimd; X-axis reduces are DVE-only and land in
        # DVE's idle window before the first obs group arrives
        d8 = gsb.tile([128, G, N, A], F32)
        nc.gpsimd.tensor_sub(d8, act4, pol4)
        dw = gsb.tile([128, G, N, A], F32)
        nc.gpsimd.tensor_mul(dw, d8, wvyb4)
        PS8 = gsb.tile([128, G, A], F32)      # sum_r pol[p,g,r,a]
        nc.vector.reduce_sum(out=PS8, in_=pol_sb.rearrange(
            "p g (r a) -> p g a r", a=8), axis=mybir.AxisListType.X)
        PSw = gsb.tile([128, G, A], F32)
        nc.gpsimd.tensor_mul(PSw, PS8, wvyb3)
        PS4 = gsb.tile([128, G], F32)
        nc.vector.reduce_sum(out=PS4, in_=PSw, axis=mybir.AxisListType.X)
        Q64 = gsb.tile([128, G, N], F32)
        nc.vector.reduce_sum(out=Q64, in_=dw, axis=mybir.AxisListType.X)
        QS4 = gsb.tile([128, G], F32)
        nc.vector.reduce_sum(out=QS4, in_=Q64, axis=mybir.AxisListType.X)

        # outputs: env e = 128g+p occupies rows 16e..16e+15
        xo_v = xo.ap().rearrange("(g p d) j -> g p (d j)", p=128, d=16)
        wo_v = wo.ap().rearrange("(g p d) j -> g p (d j)", p=128, d=16)

        def head(g):
            """reduce + transpose + MLP chain + attention scalars for group g
            (DVE: 1 op; PE: 3; ACT: 5).  Returns pwt [128,2] PSUM: col0 = w_g
            (sigmoided), col1 = c_g."""
            # node-sum: out[p, f] = sum_nf obs[p, nf*128+f]; f outer, node
            # inner so it is a single strided X-axis reduce.
            meanS = sb.tile([128, 128], F32, name="meanS")
            nc.vector.reduce_sum(
                out=meanS,
                in_=obs_tiles[g].rearrange("p (nf f) -> p f nf", nf=16),
                axis=mybir.AxisListType.X)
            pmt = pmtp.tile([128, 128], F32, name="pmt")
            nc.tensor.transpose(pmt, meanS[:], id128_sb)
            meanT = sb.tile([128, GE], F32, name="meanT")
            nc.scalar.activation(out=meanT, in_=pmt, func=AFT.Copy)
            ph = php.tile([64, GE], F32, name="ph")
            nc.tensor.matmul(ph, lhsT=w1q_sb, rhs=meanT[:], start=True, stop=True)
            h_sb = sb.tile([64, GE], F32, name="h_sb")
            nc.scalar.activation(out=h_sb, in_=ph, func=AFT.Relu, bias=b1_sb)
            pac = pacp.tile([2, GE], F32, name="pac")
            nc.tensor.matmul(pac, lhsT=wq_sb, rhs=h_sb, start=True, stop=True)
            wc = sb.tile([2, GE], F32, name="wc")
            nc.scalar.activation(out=wc, in_=pac, func=AFT.Identity,
                                 bias=biasq_sb)
            lr = sb.tile([1, GE], F32, name="lr")
            nc.vector.scalar_tensor_tensor(out=lr, in0=wc[0:1, :], scalar=0.01,
                                           in1=wc[0:1, :], op0=ALU.mult,
                                           op1=ALU.max)
            nc.scalar.activation(out=wc[0:1, :], in_=lr, func=AFT.Sigmoid)
            pwt = pwtp.tile([128, 2], F32, name="pwt")
            nc.tensor.transpose(pwt, wc[:], id2_sb)
            return pwt

        def combine_and_store(g, pwt, wo_eng, xo_eng):
            """per-group combine on DVE + broadcast payloads + output DMAs."""
            wcol = pwt[:, 0:1]
            ccol = pwt[:, 1:2]
            t2 = sb.tile([128, 1], F32, name="t2")
            nc.vector.tensor_mul(t2, wcol, QS4[:, g:g + 1])
            t3 = sb.tile([128, 1], F32, name="t3")
            nc.vector.tensor_add(t3, t2, PS4[:, g:g + 1])
            base = sb.tile([128, 1], F32, name="base")
            nc.vector.scalar_tensor_tensor(out=base, in0=t3, scalar=1.0 / N,
                                           in1=ccol, op0=ALU.mult, op1=ALU.add)
            nwq = sb.tile([128, N], F32, name="nwq")
            nc.vector.scalar_tensor_tensor(
                out=nwq, in0=Q64[:, g], scalar=-1.0 / N,
                in1=wcol.broadcast_to([128, N]), op0=ALU.mult, op1=ALU.mult)
            xv = sb.tile([128, N], F32, name="xv")
            nc.vector.tensor_add(xv, nwq, base.broadcast_to([128, N]))
            wbig = outp.tile([128, N * N], F32, name="wbig")
            nc.vector.tensor_copy(wbig, wcol.broadcast_to([128, N * N]))
            xbig = outp.tile([128, N, N], F32, name="xbig")
            nc.vector.tensor_copy(
                xbig, xv.unsqueeze(1).broadcast_to([128, N, N]))
            wo_eng.dma_start(out=wo_v[g], in_=wbig)
            xo_eng.dma_start(out=xo_v[g], in_=xbig.rearrange("p d j -> p (d j)"))

        # emission order == per-engine issue order; keep DVE as
        # r0, r1, (lr0 cg0), (lr1 cg1), r2, r3, (lr2 cg2), (lr3 cg3)
        pwt0 = head(0)
        pwt1 = head(1)
        combine_and_store(0, pwt0, wo_eng=nc.scalar, xo_eng=nc.scalar)
        combine_and_store(1, pwt1, wo_eng=nc.scalar, xo_eng=nc.scalar)
        pwt2 = head(2)
        pwt3 = head(3)
        combine_and_store(2, pwt2, wo_eng=nc.sync, xo_eng=nc.sync)
        combine_and_store(3, pwt3, wo_eng=nc.sync, xo_eng=nc.sync)

    nc.compile()
    return nc


_NC_CACHE = {}


def _get_nc():
    if "nc" not in _NC_CACHE:
        _NC_CACHE["nc"] = _build()
    return _NC_CACHE["nc"]


def _make_in_maps(inputs):
    obs = np.ascontiguousarray(np.asarray(inputs["obs"], np.float32))
    pol = np.ascontiguousarray(np.asarray(inputs["policies"], np.float32))
    act = np.ascontiguousarray(np.asarray(inputs["actions"], np.float32))
    W1 = np.asarray(inputs["W1"], np.float32)
    b1 = np.asarray(inputs["b1"], np.float32)
    W2 = np.asarray(inputs["W2"], np.float32)
    b2 = np.asarray(inputs["b2"], np.float32)
    Wfc = np.asarray(inputs["Wfc"], np.float32)
    Wattn = np.asarray(inputs["Wattn"], np.float32)
    Wv = np.asarray(inputs["Wv"], np.float32)
    bv = np.asarray(inputs["bv"], np.float32)

    wa = (Wfc @ (Wattn[:DZ] + Wattn[DZ:]))[:, 0]     # [64]
    wvy = Wv[DP:, 0]                                  # [8]

    wv64 = Wv[:DP, 0]
    cst = np.zeros((128, CW), np.float32)
    cst[:, 0:8] = wvy[None, :]
    cst[:, 8:72] = W1 / 16.0
    cst[0:64, 72] = W2 @ wa                  # Wq col 0
    cst[0:64, 73] = W2 @ wv64                # Wq col 1
    cst[0:64, 138] = b1
    cst[0, 140] = float(b2 @ wa)             # biasq
    cst[1, 140] = float(b2 @ wv64 + bv[0])
    cst[0:2, 142:144] = np.eye(2, dtype=np.float32)
    cst[:, 144:272] = np.eye(128, dtype=np.float32)

    in_maps = []
    for c in range(NCORES):
        in_maps.append({
            "obs": obs[c * RC:(c + 1) * RC],
            "pol": pol[c * RC:(c + 1) * RC],
            "act": act[c * RC:(c + 1) * RC],
            "cst": cst,
        })
    return in_maps


# Test-harness knobs (the grader just calls kernel() with defaults).
TRACE = False
TRACE_KWARGS = {}
LAST_RESULT = None


def kernel(**inputs):
    global LAST_RESULT
    nc = _get_nc()
    in_maps = _make_in_maps(inputs)
    res = run_bass_kernel_spmd(nc, in_maps, core_ids=list(range(NCORES)),
                               trace=TRACE, **TRACE_KWARGS)
    LAST_RESULT = res
    x = np.concatenate([r["xo"] for r in res.results], axis=0).reshape(B * N, N, 1)
    w = np.concatenate([r["wo"] for r in res.results], axis=0).reshape(B * N, N, 1)
    return x, w


# revision 11
# speedup vs baseline: 1.0121x; 1.0121x over previous
"""Trainium2 Bass kernel for nn_CriticNetwork (gnn_message_passing).

Math: the reference GNN does mean-aggregation over a complete graph with
self-loops, so every node of an env sees the identical per-env mean.  The
whole network collapses to per-env scalars:

  m_b  = mean over the 16 nodes of obs[b]                      [128]
  p_b  = relu(m_b @ W1 + b1) @ W2 + b2                         [64]
  a_b  = p_b . (Wfc @ (Wattn[:64] + Wattn[64:]))               scalar
  w_b  = sigmoid(leaky_relu(a_b, 0.01))                        scalar
  c_b  = p_b . Wv[:64] + bv                                    scalar
  Q_bj = (act[b,j]-pi[b,j]) . Wvy ;  (Wvy = Wv[64:72])
  PS_b = sum_j pi[b,j].Wvy ;  QS_b = sum_j Q_bj
  xv[b,j] = c_b + (PS_b + w_b*QS_b)/16 - (w_b/16)*Q_bj
  out x[b*16+d, j] = xv[b,j]   (independent of d)
  out w[b*16+d, j] = w_b

Sharding: data-parallel over envs, 512 envs per core x 8 cores.

Per-core layout (v3): local env e = 4*p + g (p = partition, g = group).
With this mapping every per-partition slab of pol/act/outputs is one
CONTIGUOUS HBM run (2KB / 2KB / 4KB), so each DMA is a single large
packet per partition instead of four 512B stragglers.

Engine plan (v3):
  - each obs group is split into two half-node DMAs, one per HWDGE ring,
    so both rings stream concurrently (they share the 16 SDMA engines;
    combined ~400+ GB/s) and groups complete early and evenly.
  - per-group node-sum: 4-op pairwise tensor_tensor tree on DVE (fastest
    fp32 reduction; 2 reads/cycle).
  - pol/act dot block (Q64/PS4/QS4) on GpSimd via pairwise-slice trees
    (GpSimd cannot do X-axis tensor_reduce); tiny final X-reduces on DVE.
  - leaky-relu folded into the scalar engine via AFT.Lrelu (alpha=0.01).
  - combine + output-broadcast per group: g0/g1 on GpSimd, g2/g3 on DVE.
  - outputs materialized in [128, 4*256] tiles -> ONE wo DMA (sync ring)
    + ONE xo DMA (scalar ring), 4KB contiguous per partition.
"""

import numpy as np
from contextlib import ExitStack

import concourse.bass as bass
import concourse.bacc as bacc
import concourse.tile as tile
from concourse import mybir
from concourse.bass_utils import run_bass_kernel_spmd

B, N, A = 4096, 16, 8
D_IN, H1, DP, DZ = 128, 64, 64, 64
NCORES = 8
BC = B // NCORES          # 512 envs per core
RC = BC * N               # 8192 obs rows per core
G = 4                     # env groups per core
GE = BC // G              # 128 envs per group
CW = 272                  # const tile width

F32 = mybir.dt.float32
ALU = mybir.AluOpType
AFT = mybir.ActivationFunctionType


def _build():
    nc = bacc.Bacc("TRN2", target_bir_lowering=False, debug=False)

    obs = nc.dram_tensor("obs", [RC, D_IN], F32, kind="ExternalInput")
    pol = nc.dram_tensor("pol", [RC, A], F32, kind="ExternalInput")
    act = nc.dram_tensor("act", [RC, A], F32, kind="ExternalInput")
    cst = nc.dram_tensor("cst", [128, CW], F32, kind="ExternalInput")
    xo = nc.dram_tensor("xo", [RC, N], F32, kind="ExternalOutput")
    wo = nc.dram_tensor("wo", [RC, N], F32, kind="ExternalOutput")

    with ExitStack() as ctx:
        tc = ctx.enter_context(tile.TileContext(nc))
        consts = ctx.enter_context(tc.tile_pool(name="consts", bufs=1))
        obsp = ctx.enter_context(tc.tile_pool(name="obsp", bufs=4))
        pap = ctx.enter_context(tc.tile_pool(name="pap", bufs=1))
        gsb = ctx.enter_context(tc.tile_pool(name="gsb", bufs=1))
        sb = ctx.enter_context(tc.tile_pool(name="sb", bufs=2))
        smal = ctx.enter_context(tc.tile_pool(name="smal", bufs=2))
        outp = ctx.enter_context(tc.tile_pool(name="outp", bufs=1))
        pmtp = ctx.enter_context(tc.tile_pool(name="pmtp", bufs=2, space="PSUM"))
        php = ctx.enter_context(tc.tile_pool(name="php", bufs=2, space="PSUM"))
        pacp = ctx.enter_context(tc.tile_pool(name="pacp", bufs=2, space="PSUM"))
        pwtp = ctx.enter_context(tc.tile_pool(name="pwtp", bufs=2, space="PSUM"))

        # small inputs first on each ring (single big packet per partition)
        cst_sb = consts.tile([128, CW], F32)
        nc.scalar.dma_start(out=cst_sb, in_=cst.ap())
        wvy8_sb = cst_sb[:, 0:8]            # Wvy on all partitions
        w1q_sb = cst_sb[:, 8:72]            # W1 / 16
        wq_sb = cst_sb[0:64, 72:74]         # W2 @ [wa | Wv[:64]]
        b1_sb = cst_sb[0:64, 138:139]
        biasq_sb = cst_sb[0:2, 140:141]     # [b2.wa, b2.Wv64 + bv]
        id2_sb = cst_sb[0:2, 142:144]       # eye(2)
        id128_sb = cst_sb[:, 144:272]       # eye(128)

        # env e = 4p+g: partition p's pol rows 64p..64p+63 are contiguous
        pa_view = lambda t: t.ap().rearrange("(p g n) a -> p (g n a)",
                                             p=128, g=G, n=16)
        pol_sb = pap.tile([128, G * N * A], F32)
        nc.sync.dma_start(out=pol_sb, in_=pa_view(pol))          # ring A
        act_sb = pap.tile([128, G * N * A], F32)
        nc.scalar.dma_start(out=act_sb, in_=pa_view(act))        # ring B

        # obs rows for env 4p+g: 64p+16g .. +15; half h = 8 nodes = 4KB
        obs_v = obs.ap().rearrange("(p g h nf) f -> g h p (nf f)",
                                   p=128, g=G, h=2, nf=8)
        obs_tiles = []
        for g in range(G):
            obs_t = obsp.tile([128, 16 * 128], F32, name="obs_t")
            nc.sync.dma_start(out=obs_t[:, 0:1024], in_=obs_v[g][0])
            nc.scalar.dma_start(out=obs_t[:, 1024:2048], in_=obs_v[g][1])
            obs_tiles.append(obs_t)

        # warm the ACT tables used later (sigmoid + leaky-relu) while
        # the obs stream runs
        warm = consts.tile([1, 2], F32)
        nc.scalar.activation(out=warm[:, 0:1], in_=cst_sb[0:1, 0:1],
                             func=AFT.Sigmoid)
        nc.scalar.activation(out=warm[:, 1:2], in_=cst_sb[0:1, 0:1],
                             func=AFT.Lrelu, alpha=0.01)

        # ---- GpSimd: pol/act dot block via pairwise-slice trees ----
        # PS8[p,(g a)] = sum_r pol[p,g,r,a]: halve the r axis of (g r a)
        pv = pol_sb.rearrange("p (g x) -> p g x", g=G)      # [128,4,128]
        ps_t1 = gsb.tile([128, G, 64], F32)
        nc.gpsimd.tensor_add(ps_t1, pv[:, :, 0:64], pv[:, :, 64:128])
        ps_t2 = gsb.tile([128, G, 32], F32)
        nc.gpsimd.tensor_add(ps_t2, ps_t1[:, :, 0:32], ps_t1[:, :, 32:64])
        ps_t3 = gsb.tile([128, G, 16], F32)
        nc.gpsimd.tensor_add(ps_t3, ps_t2[:, :, 0:16], ps_t2[:, :, 16:32])
        PS8 = gsb.tile([128, G, A], F32)
        nc.gpsimd.tensor_add(PS8, ps_t3[:, :, 0:8], ps_t3[:, :, 8:16])
        PSw = gsb.tile([128, G, A], F32)
        nc.gpsimd.tensor_mul(PSw, PS8,
                             wvy8_sb.unsqueeze(1).broadcast_to([128, G, A]))
        # d8/dw: (act - pol) * wvy over all 512 lanes
        d8 = gsb.tile([128, G * N * A], F32)
        nc.gpsimd.tensor_sub(d8, act_sb, pol_sb)
        dw = gsb.tile([128, G * N, A], F32)
        nc.gpsimd.tensor_mul(dw, d8.rearrange("p (gr a) -> p gr a", a=A),
                             wvy8_sb.unsqueeze(1).broadcast_to([128, G * N, A]))
        # Q64[p,(g r)] = sum_a dw: halve the a axis
        q_t1 = gsb.tile([128, G * N, 4], F32)
        nc.gpsimd.tensor_add(q_t1, dw[:, :, 0:4], dw[:, :, 4:8])
        q_t2 = gsb.tile([128, G * N, 2], F32)
        nc.gpsimd.tensor_add(q_t2, q_t1[:, :, 0:2], q_t1[:, :, 2:4])
        Q64 = gsb.tile([128, G * N], F32)
        nc.gpsimd.tensor_add(Q64.rearrange("p (gr one) -> p gr one", one=1),
                             q_t2[:, :, 0:1], q_t2[:, :, 1:2])

        # outputs: [128, (g d j)] == HBM rows 64p+16g+d, 4KB contiguous
        wbig = outp.tile([128, G * N * N], F32)
        xbig = outp.tile([128, G * N * N], F32)

        def head(g, to_sbuf=False):
            """tree + transpose + MLP chain for group g.
            Returns (wcol, ccol): pwt PSUM columns (w sigmoided, c raw)."""
            obs_t = obs_tiles[g]
            s1 = sb.tile([128, 1024], F32, name="s1")
            nc.vector.tensor_add(s1, obs_t[:, 0:1024], obs_t[:, 1024:2048])
            s2 = sb.tile([128, 512], F32, name="s2")
            nc.vector.tensor_add(s2, s1[:, 0:512], s1[:, 512:1024])
            s3 = sb.tile([128, 256], F32, name="s3")
            nc.vector.tensor_add(s3, s2[:, 0:256], s2[:, 256:512])
            meanS = sb.tile([128, 128], F32, name="meanS")
            nc.vector.tensor_add(meanS, s3[:, 0:128], s3[:, 128:256])
            pmt = pmtp.tile([128, 128], F32, name="pmt")
            nc.tensor.transpose(pmt, meanS[:], id128_sb)
            meanT = sb.tile([128, GE], F32, name="meanT")
            nc.scalar.activation(out=meanT, in_=pmt, func=AFT.Copy)
            ph = php.tile([64, GE], F32, name="ph")
            nc.tensor.matmul(ph, lhsT=w1q_sb, rhs=meanT[:], start=True,
                             stop=True)
            h_sb = sb.tile([64, GE], F32, name="h_sb")
            nc.scalar.activation(out=h_sb, in_=ph, func=AFT.Relu, bias=b1_sb)
            pac = pacp.tile([2, GE], F32, name="pac")
            nc.tensor.matmul(pac, lhsT=wq_sb, rhs=h_sb, start=True, stop=True)
            wc = sb.tile([2, GE], F32, name="wc")
            nc.scalar.activation(out=wc, in_=pac, func=AFT.Identity,
                                 bias=biasq_sb)
            nc.scalar.activation(out=wc[0:1, :], in_=wc[0:1, :],
                                 func=AFT.Lrelu, alpha=0.01)
            nc.scalar.activation(out=wc[0:1, :], in_=wc[0:1, :],
                                 func=AFT.Sigmoid)
            pwt = pwtp.tile([128, 2], F32, name="pwt")
            nc.tensor.transpose(pwt, wc[:], id2_sb)
            if to_sbuf:
                # GpSimd combines cannot read PSUM
                wcs = sb.tile([128, 2], F32, name="wcs")
                nc.scalar.activation(out=wcs, in_=pwt, func=AFT.Copy)
                return wcs[:, 0:1], wcs[:, 1:2]
            return pwt[:, 0:1], pwt[:, 1:2]

        def combine(eng, g, wcol, ccol):
            """per-group combine + output broadcast on engine `eng`.
            Uses only tensor_tensor/tensor_copy so it runs on GpSimd too."""
            t2 = smal.tile([128, 1], F32, name="t2")
            eng.tensor_mul(t2, wcol, QS4s[:, g:g + 1])
            t3 = smal.tile([128, 1], F32, name="t3")
            eng.tensor_add(t3, t2, PS4s[:, g:g + 1])
            base = smal.tile([128, 1], F32, name="base")
            eng.tensor_add(base, t3, ccol)
            nwq = smal.tile([128, N], F32, name="nwq")
            eng.tensor_mul(nwq, Q64n[:, N * g:N * (g + 1)],
                           wcol.broadcast_to([128, N]))
            xv = smal.tile([128, N], F32, name="xv")
            eng.tensor_add(xv, nwq, base.broadcast_to([128, N]))
            eng.tensor_copy(wbig[:, 256 * g:256 * (g + 1)],
                            wcol.broadcast_to([128, 256]))
            eng.tensor_copy(
                xbig[:, 256 * g:256 * (g + 1)].rearrange("p (d j) -> p d j",
                                                         d=16),
                xv.unsqueeze(1).broadcast_to([128, 16, 16]))

        # heads for g0/g1; DVE then does the tiny final X-reduces in the
        # gap before g2 lands
        pwt0 = head(0, to_sbuf=True)
        pwt1 = head(1, to_sbuf=True)
        PS4 = gsb.tile([128, G], F32)
        nc.vector.reduce_sum(out=PS4, in_=PSw, axis=mybir.AxisListType.X)
        QS4 = gsb.tile([128, G], F32)
        nc.vector.reduce_sum(out=QS4,
                             in_=Q64.rearrange("p (g r) -> p g r", g=G),
                             axis=mybir.AxisListType.X)
        # pre-scale by 1/16 so per-group combines are pure tensor_tensor
        PS4s = gsb.tile([128, G], F32)
        nc.vector.tensor_scalar_mul(PS4s, PS4, 1.0 / N)
        QS4s = gsb.tile([128, G], F32)
        nc.vector.tensor_scalar_mul(QS4s, QS4, 1.0 / N)
        Q64n = gsb.tile([128, G * N], F32)
        nc.vector.tensor_scalar_mul(Q64n, Q64, -1.0 / N)
        pwt2 = head(2)
        pwt3 = head(3)
        combine(nc.gpsimd, 0, *pwt0)
        combine(nc.gpsimd, 1, *pwt1)
        combine(nc.vector, 2, *pwt2)
        combine(nc.vector, 3, *pwt3)

        # two batched output DMAs, one per ring; 4KB/partition contiguous
        wo_v = wo.ap().rearrange("(p g d) j -> p (g d j)", p=128, g=G, d=16)
        xo_v = xo.ap().rearrange("(p g d) j -> p (g d j)", p=128, g=G, d=16)
        nc.sync.dma_start(out=wo_v, in_=wbig)
        nc.scalar.dma_start(out=xo_v, in_=xbig)

    nc.compile()
    return nc


_NC_CACHE = {}


def _get_nc():
    if "nc" not in _NC_CACHE:
        _NC_CACHE["nc"] = _build()
    return _NC_CACHE["nc"]


def _make_in_maps(inputs):
    obs = np.ascontiguousarray(np.asarray(inputs["obs"], np.float32))
    pol = np.ascontiguousarray(np.asarray(inputs["policies"], np.float32))
    act = np.ascontiguousarray(np.asarray(inputs["actions"], np.float32))
    W1 = np.asarray(inputs["W1"], np.float32)
    b1 = np.asarray(inputs["b1"], np.float32)
    W2 = np.asarray(inputs["W2"], np.float32)
    b2 = np.asarray(inputs["b2"], np.float32)
    Wfc = np.asarray(inputs["Wfc"], np.float32)
    Wattn = np.asarray(inputs["Wattn"], np.float32)
    Wv = np.asarray(inputs["Wv"], np.float32)
    bv = np.asarray(inputs["bv"], np.float32)

    wa = (Wfc @ (Wattn[:DZ] + Wattn[DZ:]))[:, 0]     # [64]
    wvy = Wv[DP:, 0]                                  # [8]

    wv64 = Wv[:DP, 0]
    cst = np.zeros((128, CW), np.float32)
    cst[:, 0:8] = wvy[None, :]
    cst[:, 8:72] = W1 / 16.0
    cst[0:64, 72] = W2 @ wa                  # Wq col 0
    cst[0:64, 73] = W2 @ wv64                # Wq col 1
    cst[0:64, 138] = b1
    cst[0, 140] = float(b2 @ wa)             # biasq
    cst[1, 140] = float(b2 @ wv64 + bv[0])
    cst[0:2, 142:144] = np.eye(2, dtype=np.float32)
    cst[:, 144:272] = np.eye(128, dtype=np.float32)

    in_maps = []
    for c in range(NCORES):
        in_maps.append({
            "obs": obs[c * RC:(c + 1) * RC],
            "pol": pol[c * RC:(c + 1) * RC],
            "act": act[c * RC:(c + 1) * RC],
            "cst": cst,
        })
    return in_maps


# Test-harness knobs (the grader just calls kernel() with defaults).
TRACE = False
TRACE_KWARGS = {}
LAST_RESULT = None


def kernel(**inputs):
    global LAST_RESULT
    nc = _get_nc()
    in_maps = _make_in_maps(inputs)
    res = run_bass_kernel_spmd(nc, in_maps, core_ids=list(range(NCORES)),
                               trace=TRACE, **TRACE_KWARGS)
    LAST_RESULT = res
    x = np.concatenate([r["xo"] for r in res.results], axis=0).reshape(B * N, N, 1)
    w = np.concatenate([r["wo"] for r in res.results], axis=0).reshape(B * N, N, 1)
    return x, w


# revision 12
# speedup vs baseline: 1.0737x; 1.0609x over previous
"""Trainium2 Bass kernel for nn_CriticNetwork (gnn_message_passing).

Math: the reference GNN does mean-aggregation over a complete graph with
self-loops, so every node of an env sees the identical per-env mean.  The
whole network collapses to per-env scalars:

  m_b  = mean over the 16 nodes of obs[b]                      [128]
  p_b  = relu(m_b @ W1 + b1) @ W2 + b2                         [64]
  a_b  = p_b . (Wfc @ (Wattn[:64] + Wattn[64:]))               scalar
  w_b  = sigmoid(leaky_relu(a_b, 0.01))                        scalar
  c_b  = p_b . Wv[:64] + bv                                    scalar
  Q_bj = (act[b,j]-pi[b,j]) . Wvy ;  (Wvy = Wv[64:72])
  PS_b = sum_j pi[b,j].Wvy ;  QS_b = sum_j Q_bj
  xv[b,j] = c_b + (PS_b + w_b*QS_b)/16 - (w_b/16)*Q_bj
  out x[b*16+d, j] = xv[b,j]   (independent of d)
  out w[b*16+d, j] = w_b

Sharding: data-parallel over envs, 512 envs per core x 8 cores.

Per-core layout: local env e = 4*p + g (p = partition, g = group), so
every per-partition slab of pol/act/outputs is one contiguous HBM run
(2KB/2KB/4KB) -> single big DMA packets, no 512B stragglers.

Engine plan (v4):
  - obs group g streams as two half-node DMAs into separate tiles, one
    half per HWDGE ring; DVE runs a partial tree per half as it lands,
    then adds the halves.  This overlaps most of the reduction with the
    stream itself.
  - pol/act dot block (Q64/PS4/QS4) on GpSimd via pairwise-slice trees;
    tiny final X-reduces and 1/16 pre-scales on DVE.
  - leaky-relu as a DVE scalar_tensor_tensor (AFT.Lrelu lives in a
    different ACT table than Sigmoid -> table-reload thrash; avoid).
  - combines: g0/g1 on GpSimd (from an SBUF copy of pwt), g2/g3 on DVE
    (straight from PSUM).  Separate output tiles per engine pair so the
    writers never serialize against each other.
  - outputs: 4 DMAs of [128, 512] (2KB/partition contiguous), two per
    ring.
"""

import numpy as np
from contextlib import ExitStack

import concourse.bass as bass
import concourse.bacc as bacc
import concourse.tile as tile
from concourse import mybir
from concourse.bass_utils import run_bass_kernel_spmd

B, N, A = 4096, 16, 8
D_IN, H1, DP, DZ = 128, 64, 64, 64
NCORES = 8
BC = B // NCORES          # 512 envs per core
RC = BC * N               # 8192 obs rows per core
G = 4                     # env groups per core
GE = BC // G              # 128 envs per group
CW = 272                  # const tile width

F32 = mybir.dt.float32
ALU = mybir.AluOpType
AFT = mybir.ActivationFunctionType


def _build():
    nc = bacc.Bacc("TRN2", target_bir_lowering=False, debug=False)

    obs = nc.dram_tensor("obs", [RC, D_IN], F32, kind="ExternalInput")
    pol = nc.dram_tensor("pol", [RC, A], F32, kind="ExternalInput")
    act = nc.dram_tensor("act", [RC, A], F32, kind="ExternalInput")
    cst = nc.dram_tensor("cst", [128, CW], F32, kind="ExternalInput")
    xo = nc.dram_tensor("xo", [RC, N], F32, kind="ExternalOutput")
    wo = nc.dram_tensor("wo", [RC, N], F32, kind="ExternalOutput")

    with ExitStack() as ctx:
        tc = ctx.enter_context(tile.TileContext(nc))
        consts = ctx.enter_context(tc.tile_pool(name="consts", bufs=1))
        obsp = ctx.enter_context(tc.tile_pool(name="obsp", bufs=8))
        pap = ctx.enter_context(tc.tile_pool(name="pap", bufs=1))
        gsb = ctx.enter_context(tc.tile_pool(name="gsb", bufs=1))
        sb = ctx.enter_context(tc.tile_pool(name="sb", bufs=2))
        smal = ctx.enter_context(tc.tile_pool(name="smal", bufs=2))
        outp = ctx.enter_context(tc.tile_pool(name="outp", bufs=1))
        pmtp = ctx.enter_context(tc.tile_pool(name="pmtp", bufs=2, space="PSUM"))
        php = ctx.enter_context(tc.tile_pool(name="php", bufs=2, space="PSUM"))
        pacp = ctx.enter_context(tc.tile_pool(name="pacp", bufs=2, space="PSUM"))
        pwtp = ctx.enter_context(tc.tile_pool(name="pwtp", bufs=2, space="PSUM"))

        # ring A (sync): act + the h0 obs halves; ring B (scalar): cst,
        # pol + the h1 halves.  Small transfers lead so they finish first.
        act_sb = pap.tile([128, G * N * A], F32)
        pa_view = lambda t: t.ap().rearrange("(p g n) a -> p (g n a)",
                                             p=128, g=G, n=16)
        nc.sync.dma_start(out=act_sb, in_=pa_view(act))
        cst_sb = consts.tile([128, CW], F32)
        nc.scalar.dma_start(out=cst_sb, in_=cst.ap())
        pol_sb = pap.tile([128, G * N * A], F32)
        nc.scalar.dma_start(out=pol_sb, in_=pa_view(pol))

        wvy8_sb = cst_sb[:, 0:8]            # Wvy on all partitions
        w1q_sb = cst_sb[:, 8:72]            # W1 / 16
        wq_sb = cst_sb[0:64, 72:74]         # W2 @ [wa | Wv[:64]]
        b1_sb = cst_sb[0:64, 138:139]
        biasq_sb = cst_sb[0:2, 140:141]     # [b2.wa, b2.Wv64 + bv]
        id2_sb = cst_sb[0:2, 142:144]       # eye(2)
        id128_sb = cst_sb[:, 144:272]       # eye(128)

        # obs rows for env 4p+g: 64p+16g .. +15; half h = 8 nodes = 4KB
        obs_v = obs.ap().rearrange("(p g h nf) f -> g h p (nf f)",
                                   p=128, g=G, h=2, nf=8)
        obs_half = []
        for g in range(G):
            ta = obsp.tile([128, 1024], F32, name="obs_ta")
            nc.sync.dma_start(out=ta, in_=obs_v[g][0])
            tb = obsp.tile([128, 1024], F32, name="obs_tb")
            nc.scalar.dma_start(out=tb, in_=obs_v[g][1])
            obs_half.append((ta, tb))

        # preload the sigmoid ACT table while DMAs stream
        warm = consts.tile([1, 1], F32)
        nc.scalar.activation(out=warm, in_=cst_sb[0:1, 0:1], func=AFT.Sigmoid)

        # ---- GpSimd: pol/act dot block via pairwise-slice trees ----
        pv = pol_sb.rearrange("p (g x) -> p g x", g=G)      # [128,4,128]
        ps_t1 = gsb.tile([128, G, 64], F32)
        nc.gpsimd.tensor_add(ps_t1, pv[:, :, 0:64], pv[:, :, 64:128])
        ps_t2 = gsb.tile([128, G, 32], F32)
        nc.gpsimd.tensor_add(ps_t2, ps_t1[:, :, 0:32], ps_t1[:, :, 32:64])
        ps_t3 = gsb.tile([128, G, 16], F32)
        nc.gpsimd.tensor_add(ps_t3, ps_t2[:, :, 0:16], ps_t2[:, :, 16:32])
        PS8 = gsb.tile([128, G, A], F32)
        nc.gpsimd.tensor_add(PS8, ps_t3[:, :, 0:8], ps_t3[:, :, 8:16])
        PSw = gsb.tile([128, G, A], F32)
        nc.gpsimd.tensor_mul(PSw, PS8,
                             wvy8_sb.unsqueeze(1).broadcast_to([128, G, A]))
        d8 = gsb.tile([128, G * N * A], F32)
        nc.gpsimd.tensor_sub(d8, act_sb, pol_sb)
        dw = gsb.tile([128, G * N, A], F32)
        nc.gpsimd.tensor_mul(dw, d8.rearrange("p (gr a) -> p gr a", a=A),
                             wvy8_sb.unsqueeze(1).broadcast_to([128, G * N, A]))
        q_t1 = gsb.tile([128, G * N, 4], F32)
        nc.gpsimd.tensor_add(q_t1, dw[:, :, 0:4], dw[:, :, 4:8])
        q_t2 = gsb.tile([128, G * N, 2], F32)
        nc.gpsimd.tensor_add(q_t2, q_t1[:, :, 0:2], q_t1[:, :, 2:4])
        Q64 = gsb.tile([128, G * N], F32)
        nc.gpsimd.tensor_add(Q64.rearrange("p (gr one) -> p gr one", one=1),
                             q_t2[:, :, 0:1], q_t2[:, :, 1:2])

        # tiny final reduces + pre-scales on DVE (slack between trees)
        PS4 = gsb.tile([128, G], F32)
        nc.vector.reduce_sum(out=PS4, in_=PSw, axis=mybir.AxisListType.X)
        QS4 = gsb.tile([128, G], F32)
        nc.vector.reduce_sum(out=QS4,
                             in_=Q64.rearrange("p (g r) -> p g r", g=G),
                             axis=mybir.AxisListType.X)
        PS4s = gsb.tile([128, G], F32)
        nc.vector.tensor_scalar_mul(PS4s, PS4, 1.0 / N)
        QS4s = gsb.tile([128, G], F32)
        nc.vector.tensor_scalar_mul(QS4s, QS4, 1.0 / N)
        Q64n = gsb.tile([128, G * N], F32)
        nc.vector.tensor_scalar_mul(Q64n, Q64, -1.0 / N)

        # separate output tiles per engine pair (parallel writers);
        # rows (g0,g1) and (g2,g3) are each 2KB contiguous per partition
        wbigA = outp.tile([128, 2 * N * N], F32)   # g0,g1  (GpSimd)
        xbigA = outp.tile([128, 2 * N * N], F32)
        wbigB = outp.tile([128, 2 * N * N], F32)   # g2,g3  (DVE)
        xbigB = outp.tile([128, 2 * N * N], F32)

        def head(g, to_sbuf):
            """per-half partial trees + transpose + MLP chain for group g."""
            ta, tb = obs_half[g]
            sa = sb.tile([128, 128], F32, name="sa")
            sa2 = sb.tile([128, 512], F32, name="sa2")
            nc.vector.tensor_add(sa2, ta[:, 0:512], ta[:, 512:1024])
            sa3 = sb.tile([128, 256], F32, name="sa3")
            nc.vector.tensor_add(sa3, sa2[:, 0:256], sa2[:, 256:512])
            nc.vector.tensor_add(sa, sa3[:, 0:128], sa3[:, 128:256])
            sbb = sb.tile([128, 128], F32, name="sbb")
            sb2 = sb.tile([128, 512], F32, name="sb2")
            nc.vector.tensor_add(sb2, tb[:, 0:512], tb[:, 512:1024])
            sb3 = sb.tile([128, 256], F32, name="sb3")
            nc.vector.tensor_add(sb3, sb2[:, 0:256], sb2[:, 256:512])
            nc.vector.tensor_add(sbb, sb3[:, 0:128], sb3[:, 128:256])
            meanS = sb.tile([128, 128], F32, name="meanS")
            nc.vector.tensor_add(meanS, sa, sbb)

            pmt = pmtp.tile([128, 128], F32, name="pmt")
            nc.tensor.transpose(pmt, meanS[:], id128_sb)
            meanT = sb.tile([128, GE], F32, name="meanT")
            nc.scalar.activation(out=meanT, in_=pmt, func=AFT.Copy)
            ph = php.tile([64, GE], F32, name="ph")
            nc.tensor.matmul(ph, lhsT=w1q_sb, rhs=meanT[:], start=True,
                             stop=True)
            h_sb = sb.tile([64, GE], F32, name="h_sb")
            nc.scalar.activation(out=h_sb, in_=ph, func=AFT.Relu, bias=b1_sb)
            pac = pacp.tile([2, GE], F32, name="pac")
            nc.tensor.matmul(pac, lhsT=wq_sb, rhs=h_sb, start=True, stop=True)
            wc = sb.tile([2, GE], F32, name="wc")
            nc.scalar.activation(out=wc, in_=pac, func=AFT.Identity,
                                 bias=biasq_sb)
            lr = sb.tile([1, GE], F32, name="lr")
            nc.vector.scalar_tensor_tensor(out=lr, in0=wc[0:1, :], scalar=0.01,
                                           in1=wc[0:1, :], op0=ALU.mult,
                                           op1=ALU.max)
            nc.scalar.activation(out=wc[0:1, :], in_=lr, func=AFT.Sigmoid)
            pwt = pwtp.tile([128, 2], F32, name="pwt")
            nc.tensor.transpose(pwt, wc[:], id2_sb)
            if to_sbuf:
                # GpSimd combines cannot read PSUM
                wcs = sb.tile([128, 2], F32, name="wcs")
                nc.scalar.activation(out=wcs, in_=pwt, func=AFT.Copy)
                return wcs[:, 0:1], wcs[:, 1:2]
            return pwt[:, 0:1], pwt[:, 1:2]

        def combine(eng, g, wcol, ccol, wbig, xbig, half):
            """per-group combine + output broadcast on engine `eng`;
            only tensor_tensor/tensor_copy so it runs on GpSimd too."""
            t2 = smal.tile([128, 1], F32, name="t2")
            eng.tensor_mul(t2, wcol, QS4s[:, g:g + 1])
            t3 = smal.tile([128, 1], F32, name="t3")
            eng.tensor_add(t3, t2, PS4s[:, g:g + 1])
            base = smal.tile([128, 1], F32, name="base")
            eng.tensor_add(base, t3, ccol)
            nwq = smal.tile([128, N], F32, name="nwq")
            eng.tensor_mul(nwq, Q64n[:, N * g:N * (g + 1)],
                           wcol.broadcast_to([128, N]))
            xv = smal.tile([128, N], F32, name="xv")
            eng.tensor_add(xv, nwq, base.broadcast_to([128, N]))
            o = 256 * half
            eng.tensor_copy(wbig[:, o:o + 256], wcol.broadcast_to([128, 256]))
            eng.tensor_copy(
                xbig[:, o:o + 256].rearrange("p (d j) -> p d j", d=16),
                xv.unsqueeze(1).broadcast_to([128, 16, 16]))

        pwt0 = head(0, to_sbuf=True)
        pwt1 = head(1, to_sbuf=True)
        pwt2 = head(2, to_sbuf=False)
        pwt3 = head(3, to_sbuf=False)
        combine(nc.gpsimd, 0, *pwt0, wbigA, xbigA, 0)
        combine(nc.gpsimd, 1, *pwt1, wbigA, xbigA, 1)
        combine(nc.vector, 2, *pwt2, wbigB, xbigB, 0)
        combine(nc.vector, 3, *pwt3, wbigB, xbigB, 1)

        # outputs: rows (p, gpair, d): 2KB contiguous per partition; two
        # DMAs per ring
        wo_v = wo.ap().rearrange("(p h g2 d) j -> h p (g2 d j)",
                                 p=128, h=2, g2=2, d=16)
        xo_v = xo.ap().rearrange("(p h g2 d) j -> h p (g2 d j)",
                                 p=128, h=2, g2=2, d=16)
        nc.sync.dma_start(out=wo_v[0], in_=wbigA)
        nc.scalar.dma_start(out=xo_v[0], in_=xbigA)
        nc.scalar.dma_start(out=wo_v[1], in_=wbigB)
        nc.sync.dma_start(out=xo_v[1], in_=xbigB)

    nc.compile()
    return nc


_NC_CACHE = {}


def _get_nc():
    if "nc" not in _NC_CACHE:
        _NC_CACHE["nc"] = _build()
    return _NC_CACHE["nc"]


def _make_in_maps(inputs):
    obs = np.ascontiguousarray(np.asarray(inputs["obs"], np.float32))
    pol = np.ascontiguousarray(np.asarray(inputs["policies"], np.float32))
    act = np.ascontiguousarray(np.asarray(inputs["actions"], np.float32))
    W1 = np.asarray(inputs["W1"], np.float32)
    b1 = np.asarray(inputs["b1"], np.float32)
    W2 = np.asarray(inputs["W2"], np.float32)
    b2 = np.asarray(inputs["b2"], np.float32)
    Wfc = np.asarray(inputs["Wfc"], np.float32)
    Wattn = np.asarray(inputs["Wattn"], np.float32)
    Wv = np.asarray(inputs["Wv"], np.float32)
    bv = np.asarray(inputs["bv"], np.float32)

    wa = (Wfc @ (Wattn[:DZ] + Wattn[DZ:]))[:, 0]     # [64]
    wvy = Wv[DP:, 0]                                  # [8]

    wv64 = Wv[:DP, 0]
    cst = np.zeros((128, CW), np.float32)
    cst[:, 0:8] = wvy[None, :]
    cst[:, 8:72] = W1 / 16.0
    cst[0:64, 72] = W2 @ wa                  # Wq col 0
    cst[0:64, 73] = W2 @ wv64                # Wq col 1
    cst[0:64, 138] = b1
    cst[0, 140] = float(b2 @ wa)             # biasq
    cst[1, 140] = float(b2 @ wv64 + bv[0])
    cst[0:2, 142:144] = np.eye(2, dtype=np.float32)
    cst[:, 144:272] = np.eye(128, dtype=np.float32)

    in_maps = []
    for c in range(NCORES):
        in_maps.append({
            "obs": obs[c * RC:(c + 1) * RC],
            "pol": pol[c * RC:(c + 1) * RC],
            "act": act[c * RC:(c + 1) * RC],
            "cst": cst,
        })
    return in_maps


# Test-harness knobs (the grader just calls kernel() with defaults).
TRACE = False
TRACE_KWARGS = {}
LAST_RESULT = None


def kernel(**inputs):
    global LAST_RESULT
    nc = _get_nc()
    in_maps = _make_in_maps(inputs)
    res = run_bass_kernel_spmd(nc, in_maps, core_ids=list(range(NCORES)),
                               trace=TRACE, **TRACE_KWARGS)
    LAST_RESULT = res
    x = np.concatenate([r["xo"] for r in res.results], axis=0).reshape(B * N, N, 1)
    w = np.concatenate([r["wo"] for r in res.results], axis=0).reshape(B * N, N, 1)
    return x, w


# revision 14
# speedup vs baseline: 1.0934x; 1.0183x over previous
"""Trainium2 Bass kernel for nn_CriticNetwork (gnn_message_passing).

Math: the reference GNN does mean-aggregation over a complete graph with
self-loops, so every node of an env sees the identical per-env mean.  The
whole network collapses to per-env scalars:

  m_b  = mean over the 16 nodes of obs[b]                      [128]
  p_b  = relu(m_b @ W1 + b1) @ W2 + b2                         [64]
  a_b  = p_b . (Wfc @ (Wattn[:64] + Wattn[64:]))               scalar
  w_b  = sigmoid(leaky_relu(a_b, 0.01))                        scalar
  c_b  = p_b . Wv[:64] + bv                                    scalar
  Q_bj = (act[b,j]-pi[b,j]) . Wvy ;  (Wvy = Wv[64:72])
  PS_b = sum_j pi[b,j].Wvy ;  QS_b = sum_j Q_bj
  xv[b,j] = c_b + (PS_b + w_b*QS_b)/16 - (w_b/16)*Q_bj
  out x[b*16+d, j] = xv[b,j]   (independent of d)
  out w[b*16+d, j] = w_b

Sharding: data-parallel over envs, 512 envs per core x 8 cores.

Per-core layout: local env e = 4*p + g (p = partition, g = group).
Groups are processed in PAIRS (pair 0 = g0,g1; pair 1 = g2,g3): a pair's
obs rows 64p+32*pair .. +31 are one contiguous 16KB HBM run per
partition, and all compute runs pair-wide, halving instruction count
(fixed ~13.7us preamble/teardown dominates; sem hops cost ~150ns each).

Engine plan (v5):
  - per pair, obs streams as two half-node DMAs (8 nodes each), one per
    HWDGE ring; only 3-4 entries per ring -> no queue-depth stalls.
  - DVE: pairwise-tree partial sums per half (overlaps the stream), +
    tiny X-reduces/scales, + pair-1 combine.
  - GpSimd: pol/act dot block via slice trees + pair-0 combine.
  - leaky-relu via DVE scalar_tensor_tensor (AFT.Lrelu would thrash the
    ACT table against Sigmoid).
  - separate output tiles per pair/engine; 4 output DMAs (2 per ring),
    2KB contiguous per partition.
"""

import numpy as np
from contextlib import ExitStack

import concourse.bass as bass
import concourse.bacc as bacc
import concourse.tile as tile
from concourse import mybir
from concourse.bass_utils import run_bass_kernel_spmd

B, N, A = 4096, 16, 8
D_IN, H1, DP, DZ = 128, 64, 64, 64
NCORES = 8
BC = B // NCORES          # 512 envs per core
RC = BC * N               # 8192 obs rows per core
G = 4                     # env groups per core
GE = BC // G              # 128 envs per group
CW = 272                  # const tile width

F32 = mybir.dt.float32
ALU = mybir.AluOpType
AFT = mybir.ActivationFunctionType


def _build():
    nc = bacc.Bacc("TRN2", target_bir_lowering=False, debug=False)

    obs = nc.dram_tensor("obs", [RC, D_IN], F32, kind="ExternalInput")
    pol = nc.dram_tensor("pol", [RC, A], F32, kind="ExternalInput")
    act = nc.dram_tensor("act", [RC, A], F32, kind="ExternalInput")
    cst = nc.dram_tensor("cst", [128, CW], F32, kind="ExternalInput")
    xo = nc.dram_tensor("xo", [RC, N], F32, kind="ExternalOutput")
    wo = nc.dram_tensor("wo", [RC, N], F32, kind="ExternalOutput")

    with ExitStack() as ctx:
        tc = ctx.enter_context(tile.TileContext(nc))
        consts = ctx.enter_context(tc.tile_pool(name="consts", bufs=1))
        obsp = ctx.enter_context(tc.tile_pool(name="obsp", bufs=2))
        pap = ctx.enter_context(tc.tile_pool(name="pap", bufs=1))
        gsb = ctx.enter_context(tc.tile_pool(name="gsb", bufs=1))
        sb = ctx.enter_context(tc.tile_pool(name="sb", bufs=2))
        smal = ctx.enter_context(tc.tile_pool(name="smal", bufs=2))
        outp = ctx.enter_context(tc.tile_pool(name="outp", bufs=1))
        pmtp = ctx.enter_context(tc.tile_pool(name="pmtp", bufs=2, space="PSUM"))
        php = ctx.enter_context(tc.tile_pool(name="php", bufs=2, space="PSUM"))
        pacp = ctx.enter_context(tc.tile_pool(name="pacp", bufs=2, space="PSUM"))
        pwtp = ctx.enter_context(tc.tile_pool(name="pwtp", bufs=2, space="PSUM"))

        # ring A (sync): act + h0 halves; ring B (scalar): cst, pol + h1
        act_sb = pap.tile([128, G * N * A], F32)
        pa_view = lambda t: t.ap().rearrange("(p g n) a -> p (g n a)",
                                             p=128, g=G, n=16)
        nc.sync.dma_start(out=act_sb, in_=pa_view(act))
        cst_sb = consts.tile([128, CW], F32)
        nc.scalar.dma_start(out=cst_sb, in_=cst.ap())
        pol_sb = pap.tile([128, G * N * A], F32)
        nc.scalar.dma_start(out=pol_sb, in_=pa_view(pol))

        wvy8_sb = cst_sb[:, 0:8]            # Wvy on all partitions
        w1q_sb = cst_sb[:, 8:72]            # W1 / 16
        wq_sb = cst_sb[0:64, 72:74]         # W2 @ [wa | Wv[:64]]
        b1_sb = cst_sb[0:64, 138:139]
        biasq_sb = cst_sb[0:2, 140:141]     # [b2.wa, b2.Wv64 + bv]
        id2_sb = cst_sb[0:2, 142:144]       # eye(2)
        id128_sb = cst_sb[:, 144:272]       # eye(128)

        # obs row = 64p + 32pr + 16g2 + 8h + nf; pair tile free layout
        # (h, g2, nf, f): each half h is (g2 nf f) = 2048 contiguous
        obs_v = obs.ap().rearrange("(p pr g2 h nf) f -> pr h p g2 (nf f)",
                                   p=128, pr=2, g2=2, h=2, nf=8)
        pair_tiles = []
        for pr in range(2):
            t = obsp.tile([128, 4096], F32, name="pair_t")
            nc.sync.dma_start(
                out=t[:, 0:2048].rearrange("p (g2 x) -> p g2 x", g2=2),
                in_=obs_v[pr][0])
            nc.scalar.dma_start(
                out=t[:, 2048:4096].rearrange("p (g2 x) -> p g2 x", g2=2),
                in_=obs_v[pr][1])
            pair_tiles.append(t)

        # preload the sigmoid ACT table while DMAs stream
        warm = consts.tile([1, 1], F32)
        nc.scalar.activation(out=warm, in_=cst_sb[0:1, 0:1], func=AFT.Sigmoid)

        # ---- GpSimd: pol/act dot block via pairwise-slice trees ----
        pv = pol_sb.rearrange("p (g x) -> p g x", g=G)      # [128,4,128]
        ps_t1 = gsb.tile([128, G, 64], F32)
        nc.gpsimd.tensor_add(ps_t1, pv[:, :, 0:64], pv[:, :, 64:128])
        ps_t2 = gsb.tile([128, G, 32], F32)
        nc.gpsimd.tensor_add(ps_t2, ps_t1[:, :, 0:32], ps_t1[:, :, 32:64])
        ps_t3 = gsb.tile([128, G, 16], F32)
        nc.gpsimd.tensor_add(ps_t3, ps_t2[:, :, 0:16], ps_t2[:, :, 16:32])
        PS8 = gsb.tile([128, G, A], F32)
        nc.gpsimd.tensor_add(PS8, ps_t3[:, :, 0:8], ps_t3[:, :, 8:16])
        PSw = gsb.tile([128, G, A], F32)
        nc.gpsimd.tensor_mul(PSw, PS8,
                             wvy8_sb.unsqueeze(1).broadcast_to([128, G, A]))
        d8 = gsb.tile([128, G * N * A], F32)
        nc.gpsimd.tensor_sub(d8, act_sb, pol_sb)
        dw = gsb.tile([128, G * N, A], F32)
        nc.gpsimd.tensor_mul(dw, d8.rearrange("p (gr a) -> p gr a", a=A),
                             wvy8_sb.unsqueeze(1).broadcast_to([128, G * N, A]))
        q_t1 = gsb.tile([128, G * N, 4], F32)
        nc.gpsimd.tensor_add(q_t1, dw[:, :, 0:4], dw[:, :, 4:8])
        q_t2 = gsb.tile([128, G * N, 2], F32)
        nc.gpsimd.tensor_add(q_t2, q_t1[:, :, 0:2], q_t1[:, :, 2:4])
        Q64 = gsb.tile([128, G * N], F32)
        nc.gpsimd.tensor_add(Q64.rearrange("p (gr one) -> p gr one", one=1),
                             q_t2[:, :, 0:1], q_t2[:, :, 1:2])

        # tiny final reduces + pre-scales on DVE
        PS4 = gsb.tile([128, G], F32)
        nc.vector.reduce_sum(out=PS4, in_=PSw, axis=mybir.AxisListType.X)
        QS4 = gsb.tile([128, G], F32)
        nc.vector.reduce_sum(out=QS4,
                             in_=Q64.rearrange("p (g r) -> p g r", g=G),
                             axis=mybir.AxisListType.X)
        PS4s = gsb.tile([128, G], F32)
        nc.vector.tensor_scalar_mul(PS4s, PS4, 1.0 / N)
        QS4s = gsb.tile([128, G], F32)
        nc.vector.tensor_scalar_mul(QS4s, QS4, 1.0 / N)
        Q64n = gsb.tile([128, G * N], F32)
        nc.vector.tensor_scalar_mul(Q64n, Q64, -1.0 / N)

        # output payload tiles, one pair each (independent writers)
        wbigs = [outp.tile([128, 2 * N * N], F32, name=f"wbig{i}")
                 for i in range(2)]
        xbigs = [outp.tile([128, 2 * N * N], F32, name=f"xbig{i}")
                 for i in range(2)]

        def head(pr, to_sbuf):
            """pair-wide: per-half trees + transpose + MLP chain.
            Returns [128,4] (w0,c0,w1,c1) per-env scalars."""
            t = pair_tiles[pr]
            va = t[:, 0:2048].rearrange("p (g x) -> p g x", g=2)
            vb = t[:, 2048:4096].rearrange("p (g x) -> p g x", g=2)
            sa2 = sb.tile([128, 2, 512], F32, name="sa2")
            nc.vector.tensor_add(sa2, va[:, :, 0:512], va[:, :, 512:1024])
            sa3 = sb.tile([128, 2, 256], F32, name="sa3")
            nc.vector.tensor_add(sa3, sa2[:, :, 0:256], sa2[:, :, 256:512])
            sa4 = sb.tile([128, 2, 128], F32, name="sa4")
            nc.vector.tensor_add(sa4, sa3[:, :, 0:128], sa3[:, :, 128:256])
            sb2 = sb.tile([128, 2, 512], F32, name="sb2")
            nc.vector.tensor_add(sb2, vb[:, :, 0:512], vb[:, :, 512:1024])
            sb3 = sb.tile([128, 2, 256], F32, name="sb3")
            nc.vector.tensor_add(sb3, sb2[:, :, 0:256], sb2[:, :, 256:512])
            sb4 = sb.tile([128, 2, 128], F32, name="sb4")
            nc.vector.tensor_add(sb4, sb3[:, :, 0:128], sb3[:, :, 128:256])
            meanS = sb.tile([128, 256], F32, name="meanS")
            nc.vector.tensor_add(meanS.rearrange("p (g f) -> p g f", g=2),
                                 sa4, sb4)

            pmt = pmtp.tile([128, 256], F32, name="pmt")
            nc.tensor.transpose(pmt[:, 0:128], meanS[:, 0:128], id128_sb)
            nc.tensor.transpose(pmt[:, 128:256], meanS[:, 128:256], id128_sb)
            meanT = sb.tile([128, 2 * GE], F32, name="meanT")
            nc.scalar.activation(out=meanT, in_=pmt, func=AFT.Copy)
            ph = php.tile([64, 2 * GE], F32, name="ph")
            nc.tensor.matmul(ph, lhsT=w1q_sb, rhs=meanT[:], start=True,
                             stop=True)
            h_sb = sb.tile([64, 2 * GE], F32, name="h_sb")
            nc.scalar.activation(out=h_sb, in_=ph, func=AFT.Relu, bias=b1_sb)
            pac = pacp.tile([2, 2 * GE], F32, name="pac")
            nc.tensor.matmul(pac, lhsT=wq_sb, rhs=h_sb, start=True, stop=True)
            wc = sb.tile([2, 2 * GE], F32, name="wc")
            nc.scalar.activation(out=wc, in_=pac, func=AFT.Identity,
                                 bias=biasq_sb)
            lr = sb.tile([1, 2 * GE], F32, name="lr")
            nc.vector.scalar_tensor_tensor(out=lr, in0=wc[0:1, :], scalar=0.01,
                                           in1=wc[0:1, :], op0=ALU.mult,
                                           op1=ALU.max)
            nc.scalar.activation(out=wc[0:1, :], in_=lr, func=AFT.Sigmoid)
            pwt = pwtp.tile([128, 4], F32, name="pwt")
            nc.tensor.transpose(pwt[:, 0:2], wc[:, 0:128], id2_sb)
            nc.tensor.transpose(pwt[:, 2:4], wc[:, 128:256], id2_sb)
            if to_sbuf:
                # GpSimd combine cannot read PSUM
                wcs = sb.tile([128, 4], F32, name="wcs")
                nc.scalar.activation(out=wcs, in_=pwt, func=AFT.Copy)
                return wcs
            return pwt

        def combine(eng, pr, wc4, wbig, xbig):
            """pair-wide combine + output broadcast; tensor_tensor/copy
            only, so it runs on GpSimd too."""
            w2 = wc4.rearrange("p (g two) -> p g two", two=2)[:, :, 0:1]
            c2 = wc4.rearrange("p (g two) -> p g two", two=2)[:, :, 1:2]
            t2 = smal.tile([128, 2, 1], F32, name="t2")
            eng.tensor_mul(t2, w2,
                           QS4s[:, 2 * pr:2 * pr + 2].unsqueeze(2))
            t3 = smal.tile([128, 2, 1], F32, name="t3")
            eng.tensor_add(t3, t2,
                           PS4s[:, 2 * pr:2 * pr + 2].unsqueeze(2))
            base = smal.tile([128, 2, 1], F32, name="base")
            eng.tensor_add(base, t3, c2)
            nwq = smal.tile([128, 2, N], F32, name="nwq")
            eng.tensor_mul(nwq,
                           Q64n[:, 32 * pr:32 * (pr + 1)].rearrange(
                               "p (g r) -> p g r", g=2),
                           w2.broadcast_to([128, 2, N]))
            xv = smal.tile([128, 2, N], F32, name="xv")
            eng.tensor_add(xv, nwq, base.broadcast_to([128, 2, N]))
            eng.tensor_copy(wbig.rearrange("p (g dj) -> p g dj", g=2),
                            w2.broadcast_to([128, 2, 256]))
            eng.tensor_copy(
                xbig.rearrange("p (g d j) -> p g d j", g=2, d=16),
                xv.unsqueeze(2).broadcast_to([128, 2, 16, 16]))

        wc01 = head(0, to_sbuf=True)
        wc23 = head(1, to_sbuf=False)
        combine(nc.gpsimd, 0, wc01, wbigs[0], xbigs[0])
        combine(nc.vector, 1, wc23, wbigs[1], xbigs[1])

        # outputs: rows (p, pr, g2, d); 2KB contiguous per partition
        wo_v = wo.ap().rearrange("(p h g2 d) j -> h p (g2 d j)",
                                 p=128, h=2, g2=2, d=16)
        xo_v = xo.ap().rearrange("(p h g2 d) j -> h p (g2 d j)",
                                 p=128, h=2, g2=2, d=16)
        nc.sync.dma_start(out=wo_v[0], in_=wbigs[0])
        nc.scalar.dma_start(out=xo_v[0], in_=xbigs[0])
        nc.scalar.dma_start(out=wo_v[1], in_=wbigs[1])
        nc.sync.dma_start(out=xo_v[1], in_=xbigs[1])

    nc.compile()
    return nc


_NC_CACHE = {}


def _get_nc():
    if "nc" not in _NC_CACHE:
        _NC_CACHE["nc"] = _build()
    return _NC_CACHE["nc"]


def _make_in_maps(inputs):
    obs = np.ascontiguousarray(np.asarray(inputs["obs"], np.float32))
    pol = np.ascontiguousarray(np.asarray(inputs["policies"], np.float32))
    act = np.ascontiguousarray(np.asarray(inputs["actions"], np.float32))
    W1 = np.asarray(inputs["W1"], np.float32)
    b1 = np.asarray(inputs["b1"], np.float32)
    W2 = np.asarray(inputs["W2"], np.float32)
    b2 = np.asarray(inputs["b2"], np.float32)
    Wfc = np.asarray(inputs["Wfc"], np.float32)
    Wattn = np.asarray(inputs["Wattn"], np.float32)
    Wv = np.asarray(inputs["Wv"], np.float32)
    bv = np.asarray(inputs["bv"], np.float32)

    wa = (Wfc @ (Wattn[:DZ] + Wattn[DZ:]))[:, 0]     # [64]
    wvy = Wv[DP:, 0]                                  # [8]

    wv64 = Wv[:DP, 0]
    cst = np.zeros((128, CW), np.float32)
    cst[:, 0:8] = wvy[None, :]
    cst[:, 8:72] = W1 / 16.0
    cst[0:64, 72] = W2 @ wa                  # Wq col 0
    cst[0:64, 73] = W2 @ wv64                # Wq col 1
    cst[0:64, 138] = b1
    cst[0, 140] = float(b2 @ wa)             # biasq
    cst[1, 140] = float(b2 @ wv64 + bv[0])
    cst[0:2, 142:144] = np.eye(2, dtype=np.float32)
    cst[:, 144:272] = np.eye(128, dtype=np.float32)

    in_maps = []
    for c in range(NCORES):
        in_maps.append({
            "obs": obs[c * RC:(c + 1) * RC],
            "pol": pol[c * RC:(c + 1) * RC],
            "act": act[c * RC:(c + 1) * RC],
            "cst": cst,
        })
    return in_maps


# Test-harness knobs (the grader just calls kernel() with defaults).
TRACE = False
TRACE_KWARGS = {}
LAST_RESULT = None


def kernel(**inputs):
    global LAST_RESULT
    nc = _get_nc()
    in_maps = _make_in_maps(inputs)
    res = run_bass_kernel_spmd(nc, in_maps, core_ids=list(range(NCORES)),
                               trace=TRACE, **TRACE_KWARGS)
    LAST_RESULT = res
    x = np.concatenate([r["xo"] for r in res.results], axis=0).reshape(B * N, N, 1)
    w = np.concatenate([r["wo"] for r in res.results], axis=0).reshape(B * N, N, 1)
    return x, w


# revision 16
# speedup vs baseline: 1.2713x; 1.1627x over previous
"""Trainium2 Bass kernel for nn_CriticNetwork (gnn_message_passing).

Math: the reference GNN does mean-aggregation over a complete graph with
self-loops, so every node of an env sees the identical per-env mean.  The
whole network collapses to per-env scalars:

  m_b  = mean over the 16 nodes of obs[b]                      [128]
  p_b  = relu(m_b @ W1 + b1) @ W2 + b2                         [64]
  a_b  = p_b . (Wfc @ (Wattn[:64] + Wattn[64:]))               scalar
  w_b  = sigmoid(leaky_relu(a_b, 0.01))                        scalar
  c_b  = p_b . Wv[:64] + bv                                    scalar
  Q_bj = (act[b,j]-pi[b,j]) . Wvy ;  (Wvy = Wv[64:72])
  PS_b = sum_j pi[b,j].Wvy ;  QS_b = sum_j Q_bj
  xv[b,j] = c_b + (PS_b + w_b*QS_b)/16 - (w_b/16)*Q_bj
  out x[b*16+d, j] = xv[b,j]   (independent of d)
  out w[b*16+d, j] = w_b

Sharding: data-parallel over envs, 512 envs per core x 8 cores.

Per-core layout: local env e = 4*p + g (p = partition, g = group).
Groups are processed in PAIRS (pair 0 = g0,g1; pair 1 = g2,g3): a pair's
obs rows 64p+32*pair .. +31 are one contiguous 16KB HBM run per
partition, and all compute runs pair-wide, halving instruction count
(fixed ~13.7us preamble/teardown dominates; sem hops cost ~150ns each).

Engine plan (v5):
  - per pair, obs streams as two half-node DMAs (8 nodes each), one per
    HWDGE ring; only 3-4 entries per ring -> no queue-depth stalls.
  - DVE: pairwise-tree partial sums per half (overlaps the stream), +
    tiny X-reduces/scales, + pair-1 combine.
  - GpSimd: pol/act dot block via slice trees + pair-0 combine.
  - leaky-relu via DVE scalar_tensor_tensor (AFT.Lrelu would thrash the
    ACT table against Sigmoid).
  - separate output tiles per pair/engine; 4 output DMAs (2 per ring),
    2KB contiguous per partition.
"""

import numpy as np
from contextlib import ExitStack

import concourse.bass as bass
import concourse.bacc as bacc
import concourse.tile as tile
from concourse import mybir
from concourse.bass_utils import run_bass_kernel_spmd

B, N, A = 4096, 16, 8
D_IN, H1, DP, DZ = 128, 64, 64, 64
NCORES = 8
BC = B // NCORES          # 512 envs per core
RC = BC * N               # 8192 obs rows per core
G = 4                     # env groups per core
GE = BC // G              # 128 envs per group
CW = 272                  # const tile width

F32 = mybir.dt.float32
BF16 = mybir.dt.bfloat16
ALU = mybir.AluOpType
AFT = mybir.ActivationFunctionType


def _build():
    nc = bacc.Bacc("TRN2", target_bir_lowering=False, debug=False)

    obs = nc.dram_tensor("obs", [RC, D_IN], F32, kind="ExternalInput")
    pol = nc.dram_tensor("pol", [RC, A], F32, kind="ExternalInput")
    act = nc.dram_tensor("act", [RC, A], F32, kind="ExternalInput")
    cst = nc.dram_tensor("cst", [128, CW], F32, kind="ExternalInput")
    xo = nc.dram_tensor("xo", [RC, N], F32, kind="ExternalOutput")
    wo = nc.dram_tensor("wo", [RC, N], F32, kind="ExternalOutput")

    with ExitStack() as ctx:
        tc = ctx.enter_context(tile.TileContext(nc))
        consts = ctx.enter_context(tc.tile_pool(name="consts", bufs=1))
        obsp = ctx.enter_context(tc.tile_pool(name="obsp", bufs=2))
        pap = ctx.enter_context(tc.tile_pool(name="pap", bufs=1))
        gsb = ctx.enter_context(tc.tile_pool(name="gsb", bufs=1))
        sb = ctx.enter_context(tc.tile_pool(name="sb", bufs=2))
        smal = ctx.enter_context(tc.tile_pool(name="smal", bufs=2))
        outp = ctx.enter_context(tc.tile_pool(name="outp", bufs=1))
        pmtp = ctx.enter_context(tc.tile_pool(name="pmtp", bufs=2, space="PSUM"))
        php = ctx.enter_context(tc.tile_pool(name="php", bufs=2, space="PSUM"))
        pacp = ctx.enter_context(tc.tile_pool(name="pacp", bufs=2, space="PSUM"))
        pwtp = ctx.enter_context(tc.tile_pool(name="pwtp", bufs=2, space="PSUM"))

        # ring A (sync): act + h0 halves; ring B (scalar): cst, pol + h1
        act_sb = pap.tile([128, G * N * A], F32)
        pa_view = lambda t: t.ap().rearrange("(p g n) a -> p (g n a)",
                                             p=128, g=G, n=16)
        nc.sync.dma_start(out=act_sb, in_=pa_view(act))
        cst_sb = consts.tile([128, CW], F32)
        nc.scalar.dma_start(out=cst_sb, in_=cst.ap())
        pol_sb = pap.tile([128, G * N * A], F32)
        nc.sync.dma_start(out=pol_sb, in_=pa_view(pol))

        wvy8_sb = cst_sb[:, 0:8]            # Wvy on all partitions
        w1q_sb = cst_sb[:, 8:72]            # W1 / 16
        wq_sb = cst_sb[0:64, 72:74]         # W2 @ [wa | Wv[:64]]
        b1_sb = cst_sb[0:64, 138:139]
        biasq_sb = cst_sb[0:2, 140:141]     # [b2.wa, b2.Wv64 + bv]
        id2_sb = cst_sb[0:2, 142:144]       # eye(2)
        id128_sb = cst_sb[:, 144:272]       # eye(128)

        # obs row = 64p + 32pr + 16g2 + 8h + nf; pair tile free layout
        # (h, g2, nf, f): each half h is (g2 nf f) = 2048 contiguous
        obs_v = obs.ap().rearrange("(p pr g2 h nf) f -> pr h p g2 (nf f)",
                                   p=128, pr=2, g2=2, h=2, nf=8)
        pair_tiles = []
        for pr in range(2):
            t = obsp.tile([128, 4096], F32, name="pair_t")
            nc.sync.dma_start(
                out=t[:, 0:2048].rearrange("p (g2 x) -> p g2 x", g2=2),
                in_=obs_v[pr][0])
            nc.scalar.dma_start(
                out=t[:, 2048:4096].rearrange("p (g2 x) -> p g2 x", g2=2),
                in_=obs_v[pr][1])
            pair_tiles.append(t)

        # preload the sigmoid ACT table while DMAs stream
        warm = consts.tile([1, 1], F32)
        nc.scalar.activation(out=warm, in_=cst_sb[0:1, 0:1], func=AFT.Sigmoid)
        # bf16 copies of the chain constants (trees/chain run in bf16)
        w1q_bf = consts.tile([128, 64], BF16)
        nc.scalar.activation(out=w1q_bf, in_=w1q_sb, func=AFT.Copy)
        wq_bf = consts.tile([64, 2], BF16)
        nc.scalar.activation(out=wq_bf, in_=wq_sb, func=AFT.Copy)
        id128_bf = consts.tile([128, 128], BF16)
        nc.scalar.activation(out=id128_bf, in_=id128_sb, func=AFT.Copy)

        # ---- GpSimd: pol/act dot block via pairwise-slice trees ----
        pv = pol_sb.rearrange("p (g x) -> p g x", g=G)      # [128,4,128]
        ps_t1 = gsb.tile([128, G, 64], F32)
        nc.gpsimd.tensor_add(ps_t1, pv[:, :, 0:64], pv[:, :, 64:128])
        ps_t2 = gsb.tile([128, G, 32], F32)
        nc.gpsimd.tensor_add(ps_t2, ps_t1[:, :, 0:32], ps_t1[:, :, 32:64])
        ps_t3 = gsb.tile([128, G, 16], F32)
        nc.gpsimd.tensor_add(ps_t3, ps_t2[:, :, 0:16], ps_t2[:, :, 16:32])
        PS8 = gsb.tile([128, G, A], F32)
        nc.gpsimd.tensor_add(PS8, ps_t3[:, :, 0:8], ps_t3[:, :, 8:16])
        PSw = gsb.tile([128, G, A], F32)
        nc.gpsimd.tensor_mul(PSw, PS8,
                             wvy8_sb.unsqueeze(1).broadcast_to([128, G, A]))
        d8 = gsb.tile([128, G * N * A], F32)
        nc.gpsimd.tensor_sub(d8, act_sb, pol_sb)
        dw = gsb.tile([128, G * N, A], F32)
        nc.gpsimd.tensor_mul(dw, d8.rearrange("p (gr a) -> p gr a", a=A),
                             wvy8_sb.unsqueeze(1).broadcast_to([128, G * N, A]))
        q_t1 = gsb.tile([128, G * N, 4], F32)
        nc.gpsimd.tensor_add(q_t1, dw[:, :, 0:4], dw[:, :, 4:8])
        q_t2 = gsb.tile([128, G * N, 2], F32)
        nc.gpsimd.tensor_add(q_t2, q_t1[:, :, 0:2], q_t1[:, :, 2:4])
        Q64 = gsb.tile([128, G * N], F32)
        nc.gpsimd.tensor_add(Q64.rearrange("p (gr one) -> p gr one", one=1),
                             q_t2[:, :, 0:1], q_t2[:, :, 1:2])

        # output payload tiles, one pair each (independent writers)
        wbigs = [outp.tile([128, 2 * N * N], F32, name=f"wbig{i}")
                 for i in range(2)]
        xbigs = [outp.tile([128, 2 * N * N], F32, name=f"xbig{i}")
                 for i in range(2)]

        def head(pr, to_sbuf):
            """pair-wide: per-half trees + transpose + MLP chain.
            Returns [128,4] (w0,c0,w1,c1) per-env scalars."""
            t = pair_tiles[pr]
            va = t[:, 0:2048].rearrange("p (g x) -> p g x", g=2)
            vb = t[:, 2048:4096].rearrange("p (g x) -> p g x", g=2)
            sa2 = sb.tile([128, 2, 512], BF16, name="sa2")
            nc.vector.tensor_add(sa2, va[:, :, 0:512], va[:, :, 512:1024])
            sa3 = sb.tile([128, 2, 256], BF16, name="sa3")
            nc.vector.tensor_add(sa3, sa2[:, :, 0:256], sa2[:, :, 256:512])
            sa4 = sb.tile([128, 2, 128], BF16, name="sa4")
            nc.vector.tensor_add(sa4, sa3[:, :, 0:128], sa3[:, :, 128:256])
            sb2 = sb.tile([128, 2, 512], BF16, name="sb2")
            nc.vector.tensor_add(sb2, vb[:, :, 0:512], vb[:, :, 512:1024])
            sb3 = sb.tile([128, 2, 256], BF16, name="sb3")
            nc.vector.tensor_add(sb3, sb2[:, :, 0:256], sb2[:, :, 256:512])
            sb4 = sb.tile([128, 2, 128], BF16, name="sb4")
            nc.vector.tensor_add(sb4, sb3[:, :, 0:128], sb3[:, :, 128:256])
            meanS = sb.tile([128, 256], BF16, name="meanS")
            nc.vector.tensor_add(meanS.rearrange("p (g f) -> p g f", g=2),
                                 sa4, sb4)

            pmt = pmtp.tile([128, 256], BF16, name="pmt")
            nc.tensor.transpose(pmt[:, 0:128], meanS[:, 0:128], id128_bf)
            nc.tensor.transpose(pmt[:, 128:256], meanS[:, 128:256], id128_bf)
            meanT = sb.tile([128, 2 * GE], BF16, name="meanT")
            nc.scalar.activation(out=meanT, in_=pmt, func=AFT.Copy)
            ph = php.tile([64, 2 * GE], F32, name="ph")
            nc.tensor.matmul(ph, lhsT=w1q_bf[:], rhs=meanT[:], start=True,
                             stop=True)
            h_sb = sb.tile([64, 2 * GE], BF16, name="h_sb")
            nc.scalar.activation(out=h_sb, in_=ph, func=AFT.Relu, bias=b1_sb)
            pac = pacp.tile([2, 2 * GE], F32, name="pac")
            nc.tensor.matmul(pac, lhsT=wq_bf[:], rhs=h_sb, start=True, stop=True)
            wc = sb.tile([2, 2 * GE], F32, name="wc")
            nc.scalar.activation(out=wc, in_=pac, func=AFT.Identity,
                                 bias=biasq_sb)
            lr = sb.tile([1, 2 * GE], F32, name="lr")
            nc.vector.scalar_tensor_tensor(out=lr, in0=wc[0:1, :], scalar=0.01,
                                           in1=wc[0:1, :], op0=ALU.mult,
                                           op1=ALU.max)
            nc.scalar.activation(out=wc[0:1, :], in_=lr, func=AFT.Sigmoid)
            pwt = pwtp.tile([128, 4], F32, name="pwt")
            nc.tensor.transpose(pwt[:, 0:2], wc[:, 0:128], id2_sb)
            nc.tensor.transpose(pwt[:, 2:4], wc[:, 128:256], id2_sb)
            if to_sbuf:
                # GpSimd combine cannot read PSUM
                wcs = sb.tile([128, 4], F32, name="wcs")
                nc.scalar.activation(out=wcs, in_=pwt, func=AFT.Copy)
                return wcs
            return pwt

        def combine(eng, pr, wc4, wbig, xbig):
            """pair-wide combine + output broadcast; tensor_tensor/copy
            only, so it runs on GpSimd too."""
            w2 = wc4.rearrange("p (g two) -> p g two", two=2)[:, :, 0:1]
            c2 = wc4.rearrange("p (g two) -> p g two", two=2)[:, :, 1:2]
            t2 = smal.tile([128, 2, 1], F32, name="t2")
            eng.tensor_mul(t2, w2,
                           QS4s[:, 2 * pr:2 * pr + 2].unsqueeze(2))
            t3 = smal.tile([128, 2, 1], F32, name="t3")
            eng.tensor_add(t3, t2,
                           PS4s[:, 2 * pr:2 * pr + 2].unsqueeze(2))
            base = smal.tile([128, 2, 1], F32, name="base")
            eng.tensor_add(base, t3, c2)
            nwq = smal.tile([128, 2, N], F32, name="nwq")
            eng.tensor_mul(nwq,
                           Q64n[:, 32 * pr:32 * (pr + 1)].rearrange(
                               "p (g r) -> p g r", g=2),
                           w2.broadcast_to([128, 2, N]))
            xv = smal.tile([128, 2, N], F32, name="xv")
            eng.tensor_add(xv, nwq, base.broadcast_to([128, 2, N]))
            eng.tensor_copy(wbig.rearrange("p (g dj) -> p g dj", g=2),
                            w2.broadcast_to([128, 2, 256]))
            eng.tensor_copy(
                xbig.rearrange("p (g d j) -> p g d j", g=2, d=16),
                xv.unsqueeze(2).broadcast_to([128, 2, 16, 16]))

        wc01 = head(0, to_sbuf=True)
        wc23 = head(1, to_sbuf=False)
        # tiny final reduces + pre-scales on DVE
        PS4 = gsb.tile([128, G], F32)
        nc.vector.reduce_sum(out=PS4, in_=PSw, axis=mybir.AxisListType.X)
        QS4 = gsb.tile([128, G], F32)
        nc.vector.reduce_sum(out=QS4,
                             in_=Q64.rearrange("p (g r) -> p g r", g=G),
                             axis=mybir.AxisListType.X)
        PS4s = gsb.tile([128, G], F32)
        nc.vector.tensor_scalar_mul(PS4s, PS4, 1.0 / N)
        QS4s = gsb.tile([128, G], F32)
        nc.vector.tensor_scalar_mul(QS4s, QS4, 1.0 / N)
        Q64n = gsb.tile([128, G * N], F32)
        nc.vector.tensor_scalar_mul(Q64n, Q64, -1.0 / N)

        combine(nc.gpsimd, 0, wc01, wbigs[0], xbigs[0])
        combine(nc.vector, 1, wc23, wbigs[1], xbigs[1])

        # outputs: rows (p, pr, g2, d); 2KB contiguous per partition
        wo_v = wo.ap().rearrange("(p h g2 d) j -> h p (g2 d j)",
                                 p=128, h=2, g2=2, d=16)
        xo_v = xo.ap().rearrange("(p h g2 d) j -> h p (g2 d j)",
                                 p=128, h=2, g2=2, d=16)
        nc.sync.dma_start(out=wo_v[0], in_=wbigs[0])
        nc.scalar.dma_start(out=xo_v[0], in_=xbigs[0])
        nc.scalar.dma_start(out=wo_v[1], in_=wbigs[1])
        nc.sync.dma_start(out=xo_v[1], in_=xbigs[1])

    nc.compile()
    return nc


_NC_CACHE = {}


def _get_nc():
    if "nc" not in _NC_CACHE:
        _NC_CACHE["nc"] = _build()
    return _NC_CACHE["nc"]


def _make_in_maps(inputs):
    obs = np.ascontiguousarray(np.asarray(inputs["obs"], np.float32))
    pol = np.ascontiguousarray(np.asarray(inputs["policies"], np.float32))
    act = np.ascontiguousarray(np.asarray(inputs["actions"], np.float32))
    W1 = np.asarray(inputs["W1"], np.float32)
    b1 = np.asarray(inputs["b1"], np.float32)
    W2 = np.asarray(inputs["W2"], np.float32)
    b2 = np.asarray(inputs["b2"], np.float32)
    Wfc = np.asarray(inputs["Wfc"], np.float32)
    Wattn = np.asarray(inputs["Wattn"], np.float32)
    Wv = np.asarray(inputs["Wv"], np.float32)
    bv = np.asarray(inputs["bv"], np.float32)

    wa = (Wfc @ (Wattn[:DZ] + Wattn[DZ:]))[:, 0]     # [64]
    wvy = Wv[DP:, 0]                                  # [8]

    wv64 = Wv[:DP, 0]
    cst = np.zeros((128, CW), np.float32)
    cst[:, 0:8] = wvy[None, :]
    cst[:, 8:72] = W1 / 16.0
    cst[0:64, 72] = W2 @ wa                  # Wq col 0
    cst[0:64, 73] = W2 @ wv64                # Wq col 1
    cst[0:64, 138] = b1
    cst[0, 140] = float(b2 @ wa)             # biasq
    cst[1, 140] = float(b2 @ wv64 + bv[0])
    cst[0:2, 142:144] = np.eye(2, dtype=np.float32)
    cst[:, 144:272] = np.eye(128, dtype=np.float32)

    in_maps = []
    for c in range(NCORES):
        in_maps.append({
            "obs": obs[c * RC:(c + 1) * RC],
            "pol": pol[c * RC:(c + 1) * RC],
            "act": act[c * RC:(c + 1) * RC],
            "cst": cst,
        })
    return in_maps


# Test-harness knobs (the grader just calls kernel() with defaults).
TRACE = False
TRACE_KWARGS = {}
LAST_RESULT = None


def kernel(**inputs):
    global LAST_RESULT
    nc = _get_nc()
    in_maps = _make_in_maps(inputs)
    res = run_bass_kernel_spmd(nc, in_maps, core_ids=list(range(NCORES)),
                               trace=TRACE, **TRACE_KWARGS)
    LAST_RESULT = res
    x = np.concatenate([r["xo"] for r in res.results], axis=0).reshape(B * N, N, 1)
    w = np.concatenate([r["wo"] for r in res.results], axis=0).reshape(B * N, N, 1)
    return x, w
